# revision 77
# baseline (speedup 1.0000x reference)
"""Trainium2 Bass kernel for nn_CleanAttention (sliding-window GQA attention).

Problem: x[2,4096,2048] -> qkv proj -> rope -> sliding-window (256) attention
(16 q heads, 4 kv heads, d=128) -> o proj.

Sharding: 8 cores = batch(2) x token-quarters(4). Each core computes all 16
heads for its 1024 tokens, using a 256-token key/value halo on the left.
Outputs concatenate: no inter-core reduction.

v3 dataflow (per core): all four projections run as split-fp8 DoubleRow
matmuls (operands held as fp8e4m3 hi+lo pairs; out = hi*hi + hi*lo + lo*hi,
each term a K=256-per-instruction DoubleRow chain at 4x bf16 throughput =>
net 0.75x the bf16 cost with ~bf16 accuracy). Weights are globally scaled to
sigma~1 host-side (SW=sqrt(2048)) so the fp8 lo-residuals stay out of the
denormal floor; the scales fold into the exp scale, the V evacuation
(SV/SW), and the final output copy (1/(SV*SW)). x and all weights are split
host-side for free; yt is split on-device (DVE mul -> ACT fp8 cast -> DVE
residual via scalar_tensor_tensor).

  x_hi/lo   fp8 [128p, 16kc, 1280t] resident
  V   = x @ wv.T   10 x [128t, 512hd] bf16 (ACT evac scales by SV/SW)
  K^T = wk_p @ x.T (+rope)   4 x [128d, 1280t] bf16
  Q^T = wq_p @ x.T (+rope)   per (half,g): [128d, 4h, 512t] bf16
  attention (i = (g,chunk)), software-pipelined `skew` deep, bf16 matmuls:
    S^T(i) = K^T_blk @ Q^T; E^T = exp(S^T * SCALE/SW^2) (ACT);
    binmasks on Pool; sums/outT matmuls; rec = 1/sums (DVE);
    ytf = outT*rec (DVE bf16) -> yth fp8 (ACT) + ytl fp8 (DVE)
  o = yt @ wo.T as DoubleRow pairs of heads; half 0's o-proj is chopped into
  16 (csx,lc) chunks interleaved one-two per attention iteration of half 1
  (fills PE bubbles left by the ACT/DVE/Pool support chain); half 1's runs
  at the end with wo prefetched one csx ahead.

DMA: everything except nothing rides the SP HWDGE queue in PE-consumption
order (the ACT queue head-of-line-blocks its own activations); x moves in
>=512B-run chunks; wq/wk hi+lo packed per head/group for one DMA each; wo in
4 tiles per csx.

RoPE: wq/wk rows host-permuted per head to [even dims | odd dims]; rotation
is 6 DVE ops on bf16; Q-side tables are stride-0 broadcasts of slices of the
K tables (no separate Q tables).
Boundary masking: halo x is zero => K=0 => scores=0 => exp=1; binmasks zero
the kb0/causal planes, and the unmasked kb1 plane of chunk 0 is corrected by
subtracting 128 from the softmax denominator (corr plane, nonzero only on
cores 0 and 4).
"""

import math

import numpy as np
from ml_dtypes import bfloat16

import concourse.bass as bass
import concourse.mybir as mybir
import concourse.tile as tile
from concourse import bacc
from concourse import bass_utils

B, T, C = 2, 4096, 2048
NH, NKV, D = 16, 4, 128
WINDOW = 256
N_CORES = 8
TCORE = 1024  # own tokens per core
HALO = 256
TX = TCORE + HALO  # 1280
NG = 4  # kv groups
GH = 4  # q heads per group
NCHUNK = 8  # query chunks of 128 per core
SCALE = 1.0 / math.sqrt(D)

f32 = mybir.dt.float32
bf16 = mybir.dt.bfloat16
fp8 = mybir.dt.float8e4
np8 = mybir.dt.np(fp8)
DR = mybir.MatmulPerfMode.DoubleRow

# Global operand scaling for fp8: weights scaled to sigma~1 (x already is).
# wq,wk scaled by SW => scores scaled SW^2, folded into the exp scale.
# wv scaled by SW => V-evac rescales by SV/SW so yt = SV*y (sigma~1 for its
# fp8 split); wo scaled by SW for its split; final evac divides by SV*SW.
SW = float(np.sqrt(2048.0))
SV = 16.0
SCALE8 = SCALE / (SW * SW)

_CACHE = {}


def _split8(a):
    """Return (hi, lo) float8_e4m3 split of array a (hi+lo ~= a)."""
    hi = a.astype(np8)
    lo = (a.astype(np.float32) - hi.astype(np.float32)).astype(np8)
    return hi, lo


def _build_nc(repeat=1, st_bufs=6, so_bufs=2, et_bufs=5, wq_bufs=4, wo_bufs=9,
              rt_bufs=3, rawq_bufs=2, yt_bufs=19, osb_bufs=4, skew=4, mpt=16,
              ytl_dve=True, wo_merge=True, ph0_pool=False, qraw_pool=False,
              e01t=0, wq_act=False, wo_bufs2=3):
    nc = bacc.Bacc("TRN2", target_bir_lowering=False, debug=False)

    xt_hi = nc.dram_tensor("xt_hi", [128, 16, TX], fp8, kind="ExternalInput")
    xt_lo = nc.dram_tensor("xt_lo", [128, 16, TX], fp8, kind="ExternalInput")
    # hi/lo splits packed on axis 1 => one DMA per head/group
    wq_t = nc.dram_tensor("wq_t", [NH, 128, 2, 16, 128], fp8, kind="ExternalInput")
    wk_t = nc.dram_tensor("wk_t", [NKV, 128, 2, 16, 128], fp8, kind="ExternalInput")
    wv_hi = nc.dram_tensor("wv_hi", [128, 16, 512], fp8, kind="ExternalInput")
    wv_lo = nc.dram_tensor("wv_lo", [128, 16, 512], fp8, kind="ExternalInput")
    wo_t = nc.dram_tensor("wo_t", [16, 4, 128, 2, 512], fp8, kind="ExternalInput")
    cs_t = nc.dram_tensor("cs_t", [2, 128, TX], bf16, kind="ExternalInput")
    bmp_t = nc.dram_tensor("bmp_t", [128, 3, 2, 512], bf16, kind="ExternalInput")
    corr_t = nc.dram_tensor("corr_t", [128, 512], f32, kind="ExternalInput")
    ones_in = nc.dram_tensor("ones_in", [128, 128], bf16, kind="ExternalInput")
    o_out = nc.dram_tensor("o_out", [TCORE, C], bf16, kind="ExternalOutput")

    exp_t = mybir.ActivationFunctionType.Exp
    copy_t = mybir.ActivationFunctionType.Copy

    with tile.TileContext(nc) as tc:
        with (
            tc.sbuf_pool(name="fixed", bufs=1) as fixed,
            tc.sbuf_pool(name="xtp", bufs=1) as xtp,
            tc.sbuf_pool(name="ktp", bufs=1) as ktp,
            tc.sbuf_pool(name="vp", bufs=1) as vp,
            tc.sbuf_pool(name="ropetmp", bufs=1) as ropetmp,
            tc.sbuf_pool(name="wqp", bufs=wq_bufs) as wqp,
            tc.psum_pool(name="ps", bufs=st_bufs) as ps,
        ):
            # --- fixed small tables (order = DMA priority; compute-critical
            # loads for phase 0 are issued inside the rep loop before these
            # on the first pass via emission order) ---
            cos_sb = fixed.tile([128, TX], bf16)
            sin_sb = fixed.tile([128, TX], bf16)
            bmp_sb = fixed.tile([128, 3, 2, 512], bf16)
            corr_sb = fixed.tile([128, 512], f32)
            ones_sb = fixed.tile([128, 128], bf16)

            def load_fixed():
                # on the SP queue AFTER the phase-0 critical feed: queue
                # order (not emission position) is what the scheduler keeps
                nc.sync.dma_start(cos_sb[:], cs_t[0])
                nc.sync.dma_start(sin_sb[:], cs_t[1])
                nc.sync.dma_start(bmp_sb[:], bmp_t[:])
                nc.sync.dma_start(corr_sb[:], corr_t[:])
                nc.sync.dma_start(ones_sb[:], ones_in[:])

            def rope(dst, raw, c1_ap, c2_ap, width):
                # raw rows [0:64]=even dims e, [64:128]=odd dims o (bf16 sbuf)
                # c1 = [cos (top) | sin (bottom)], c2 = [sin | cos]
                # dst[0:64] = e*cos - o*sin ; dst[64:128] = e*sin + o*cos
                # (tensor_tensor inputs must share the start partition, so
                # each product lands in a base-0 temp tile first)
                t1 = ropetmp.tile([64, 2048], bf16, name="t1", tag="rt",
                                  bufs=rt_bufs)
                t2 = ropetmp.tile([64, 2048], bf16, name="t2", tag="rt",
                                  bufs=rt_bufs)
                nc.vector.tensor_mul(t1[:, :width], raw[0:64], c1_ap[0:64])
                nc.vector.tensor_mul(t2[:, :width], raw[64:128], c1_ap[64:128])
                nc.vector.tensor_sub(dst[0:64], t1[:, :width], t2[:, :width])
                t3 = ropetmp.tile([64, 2048], bf16, name="t3", tag="rt",
                                  bufs=rt_bufs)
                t4 = ropetmp.tile([64, 2048], bf16, name="t4", tag="rt",
                                  bufs=rt_bufs)
                nc.vector.tensor_mul(t3[:, :width], raw[0:64], c2_ap[0:64])
                nc.vector.tensor_mul(t4[:, :width], raw[64:128], c2_ap[64:128])
                nc.vector.tensor_add(dst[64:128], t3[:, :width], t4[:, :width])

            def rope_q(dst, raw, sl):
                # dst/raw: [128, 4, 512] APs; rope tables are slices of the
                # K tables broadcast (stride-0) over the 4-head dim
                def bc(tab, p0):
                    return tab[p0 : p0 + 64, sl][:, None, :].to_broadcast(
                        [64, 4, 512])

                ts_ = [ropetmp.tile([64, 2048], bf16, name=f"t{n}q", tag="rt",
                                    bufs=rt_bufs) for n in range(4)]
                tv = [t[:].rearrange("p (a b) -> p a b", a=4) for t in ts_]
                nc.vector.tensor_mul(tv[0], raw[0:64], bc(cos_sb, 0))
                nc.vector.tensor_mul(tv[1], raw[64:128], bc(cos_sb, 64))
                nc.vector.tensor_sub(dst[0:64], tv[0], tv[1])
                nc.vector.tensor_mul(tv[2], raw[0:64], bc(sin_sb, 0))
                nc.vector.tensor_mul(tv[3], raw[64:128], bc(sin_sb, 64))
                nc.vector.tensor_add(dst[64:128], tv[2], tv[3])

            for rep in range(repeat):
                xh_sb = xtp.tile([128, 16, TX], fp8, name="xh_sb", tag="xt")
                xl_sb = xtp.tile([128, 16, TX], fp8, name="xl_sb", tag="xtl")
                kt_tiles = [
                    ktp.tile([128, TX], bf16, name=f"ktg{g}", tag=f"ktg{g}")
                    for g in range(NG)
                ]
                v_tiles = [
                    vp.tile([128, 512], bf16, name=f"vtb{tb}", tag=f"vtb{tb}")
                    for tb in range(10)
                ]

                # ---- phase 0: V and K projections ----
                with (
                    tc.sbuf_pool(name="wvp", bufs=1) as wvp,
                    tc.sbuf_pool(name="wkp", bufs=2) as wkp,
                    tc.sbuf_pool(name="krawp", bufs=4) as krawp,
                ):
                    # DMA priority order: hi operands + first x chunks feed
                    # the first matmul terms; lo operands follow; tables and
                    # Q-side constants come later.
                    wvh_sb = wvp.tile([128, 16, 512], fp8, name="wvh")
                    wvl_sb = wvp.tile([128, 16, 512], fp8, name="wvl")
                    # SP queue in PE consumption order (kproj first: each wk
                    # DMA is 728ns per 2.6us of PE work unlocked); few large
                    # DMAs since HWDGE costs 625ns fixed per transfer
                    wk_sbs = []
                    for g in range(NG):
                        wk_sbs.append(wkp.tile([128, 2, 16, 128], fp8,
                                               name=f"wkg{g}", tag="wk",
                                               bufs=NG))
                    nc.sync.dma_start(xh_sb[:, :, 0:128], xt_hi[:, :, 0:128])
                    nc.sync.dma_start(wk_sbs[0][:, 0], wk_t[0, :, 0])
                    nc.sync.dma_start(wk_sbs[0][:, 1], wk_t[0, :, 1])
                    for g in range(1, NG):
                        nc.sync.dma_start(wk_sbs[g][:], wk_t[g])
                    nc.sync.dma_start(xl_sb[:, :, 0:128], xt_lo[:, :, 0:128])
                    nc.sync.dma_start(xh_sb[:, :, 128:640], xt_hi[:, :, 128:640])
                    nc.sync.dma_start(xl_sb[:, :, 128:640], xt_lo[:, :, 128:640])
                    nc.sync.dma_start(wvh_sb[:], wv_hi[:])
                    nc.sync.dma_start(wvl_sb[:], wv_lo[:])
                    nc.sync.dma_start(xh_sb[:, :, 640:1152], xt_hi[:, :, 640:1152])
                    nc.sync.dma_start(xl_sb[:, :, 640:1152], xt_lo[:, :, 640:1152])
                    nc.sync.dma_start(xh_sb[:, :, 1152:TX], xt_hi[:, :, 1152:TX])
                    nc.sync.dma_start(xl_sb[:, :, 1152:TX], xt_lo[:, :, 1152:TX])

                    def dr_terms(p_ap, lhs_hi, lhs_lo, rhs_hi, rhs_lo, lslc, rslc):
                        """24 DoubleRow matmuls: hi*hi + hi*lo + lo*hi."""
                        terms = ((lhs_hi, rhs_hi), (lhs_hi, rhs_lo),
                                 (lhs_lo, rhs_hi))
                        for ti, (la, rb) in enumerate(terms):
                            for kk in range(8):
                                k2 = slice(2 * kk, 2 * kk + 2)
                                nc.tensor.matmul(
                                    p_ap,
                                    la[:, k2, lslc],
                                    rb[:, k2, rslc],
                                    start=(ti == 0 and kk == 0),
                                    stop=(ti == 2 and kk == 7),
                                    perf_mode=DR,
                                )

                    def vproj(tb):
                        pv = ps.tile([128, 512], f32, name=f"pv{tb}", tag="ps")
                        dr_terms(pv[:], xh_sb, xl_sb, wvh_sb, wvl_sb,
                                 slice(tb * 128, (tb + 1) * 128), slice(None))
                        if ph0_pool:
                            nc.gpsimd.tensor_scalar_mul(v_tiles[tb][:], pv[:],
                                                        SV / SW)
                        else:
                            nc.scalar.activation(v_tiles[tb][:], pv[:], copy_t,
                                                 bias=0.0, scale=SV / SW)

                    kraws = {}

                    def kproj_all(ts, tw):
                        # per-term emission across all groups, matching DMA
                        # arrival: wk_hi*xh (4g), wk_lo*xh (4g), wk_hi*xl (4g)
                        rs = slice(ts, ts + tw)
                        pks = {}
                        for g in range(NG):
                            pks[g] = ps.tile([128, 512], f32,
                                             name=f"pk{g}_{ts}", tag="ps")
                        for ti, (wi, xa) in enumerate(
                                ((0, xh_sb), (1, xh_sb), (0, xl_sb))):
                            for g in range(NG):
                                for kk in range(8):
                                    k2 = slice(2 * kk, 2 * kk + 2)
                                    nc.tensor.matmul(
                                        pks[g][:, :tw],
                                        wk_sbs[g][:, wi, k2, :],
                                        xa[:, k2, rs],
                                        start=(ti == 0 and kk == 0),
                                        stop=(ti == 2 and kk == 7),
                                        perf_mode=DR,
                                    )
                                if ti == 2:
                                    if g not in kraws:
                                        kraws[g] = krawp.tile(
                                            [128, TX], bf16,
                                            name=f"kraw{g}", tag="kraw")
                                    if ph0_pool:
                                        nc.gpsimd.tensor_copy(
                                            kraws[g][:, rs], pks[g][:, :tw])
                                    else:
                                        nc.scalar.copy(kraws[g][:, rs],
                                                       pks[g][:, :tw])

                    # interleave with xt chunk arrival order
                    kproj_all(0, 128)
                    kproj_all(128, 512)
                    for tb in (0, 1, 2, 3):
                        vproj(tb)
                    if rep == 0:
                        load_fixed()
                    kproj_all(640, 512)
                    for tb in (4, 5, 6, 7):
                        vproj(tb)
                    kproj_all(1152, 128)
                    for tb in (8, 9):
                        vproj(tb)
                    for g in range(NG):
                        rope(kt_tiles[g], kraws[g], cos_sb, sin_sb, TX)

                # ---- main: per half: Q proj + attention + O proj ----
                with (
                    tc.sbuf_pool(name="qtp", bufs=5) as qtp,
                    tc.sbuf_pool(name="ytp", bufs=yt_bufs) as ytp,
                    tc.sbuf_pool(name="wop", bufs=wo_bufs) as wop,
                    tc.sbuf_pool(name="wrk", bufs=1) as wrk,
                ):
                    def o_evac(hh, csx, lc, pos_lc):
                        osb = wrk.tile([128, 512], bf16,
                                       name=f"osb{hh}_{csx}_{lc}",
                                       tag="osb", bufs=osb_bufs)
                        nc.scalar.activation(osb[:], pos_lc[:], copy_t,
                                             bias=0.0, scale=1.0 / (SV * SW))
                        nc.sync.dma_start(
                            o_out[
                                (hh * 4 + lc) * 128 : (hh * 4 + lc) * 128 + 128,
                                csx * 512 : csx * 512 + 512,
                            ],
                            osb[:],
                        )

                    def wo_load(hh, csx):
                        wo_sbs = []
                        for q in range(4):
                            wo_sb = wop.tile([128, 2, 2, 2, 512], fp8,
                                             name=f"wo{hh}_q{q}_{csx}",
                                             tag="wo", bufs=2 * wo_bufs2)
                            nc.sync.dma_start(
                                wo_sb[:],
                                wo_t[4 * q : 4 * q + 4, csx].rearrange(
                                    "(a m) p l n -> p a m l n", m=2
                                ),
                            )
                            wo_sbs.extend(wo_sb[:, a] for a in range(2))
                        return wo_sbs

                    def o_mms(hh, hyts, wo_sbs, po, csx, lc, evac_inline):
                        # 24 DR matmuls: mp pairs heads (2mp, 2mp+1); terms
                        # y_hi*wo_hi, y_hi*wo_lo, y_lo*wo_hi
                        for ti in range(3):
                            for mp in range(8):
                                m0 = 2 * mp
                                yh, yl = hyts[(m0 // 4, lc)]
                                ya = yl if ti == 2 else yh
                                hl = 1 if ti == 1 else 0
                                nc.tensor.matmul(
                                    po[:],
                                    ya[:, (m0 % 4) * 128 : (m0 % 4) * 128 + 256]
                                    .rearrange("p (k m) -> p k m", k=2),
                                    wo_sbs[mp][:, :, hl, :],
                                    start=(ti == 0 and mp == 0),
                                    stop=(ti == 2 and mp == 7),
                                    perf_mode=DR,
                                )
                        if evac_inline:
                            o_evac(hh, csx, lc, po)

                    def oproj_chunks(hh, hyts):
                        # 16 closures, one (csx, lc) chunk each, emitted one
                        # per attention iteration of the NEXT half so the PE
                        # fills support-chain bubbles; wo prefetch 1 csx ahead
                        wo_map = {}

                        def chunk(j):
                            csx, lc = divmod(j, 4)
                            if j == 0:
                                wo_map[0] = wo_load(hh, 0)
                            if lc == 2 and csx + 1 < 4:
                                wo_map[csx + 1] = wo_load(hh, csx + 1)
                            po = ps.tile([128, 512], f32,
                                         name=f"po{hh}_{csx}_{lc}", tag="ps")
                            o_mms(hh, hyts, wo_map[csx], po, csx, lc, True)

                        return [lambda j=j: chunk(j) for j in range(16)]

                    def oproj_last(hh, hyts):
                        sbs = {0: wo_load(hh, 0)}
                        for csx in range(4):
                            if csx + 1 < 4:
                                sbs[csx + 1] = wo_load(hh, csx + 1)
                            for lc in range(4):
                                po = ps.tile([128, 512], f32,
                                             name=f"po{hh}_{csx}_{lc}",
                                             tag="ps")
                                o_mms(hh, hyts, sbs[csx], po, csx, lc, True)

                    prev_oproj = None
                    for half in range(2):
                        tok0 = 256 + half * 512  # local token offset of queries
                        iters = [(g, lc) for g in range(NG) for lc in range(4)]
                        state = {}  # i -> (et, c, g)
                        qts = []
                        yts = {}

                        def qproj(g):
                            qt_sb = qtp.tile([128, GH, 512], bf16,
                                             name=f"qt{half}_{g}", tag="qt")
                            qraw = ropetmp.tile([128, 4, 512], bf16,
                                                name=f"qraw{g}", tag="qraw",
                                                bufs=rawq_bufs)
                            for m in range(GH):
                                h = g * GH + m
                                wq_sb = wqp.tile([128, 2, 16, 128], fp8,
                                                 name=f"wqh{h}", tag="wq")
                                (nc.scalar if wq_act else nc.sync).dma_start(
                                    wq_sb[:], wq_t[h])
                                pq = ps.tile([128, 512], f32, name=f"pq{h}",
                                             tag="ps")
                                dr_terms(pq[:], wq_sb[:, 0], wq_sb[:, 1],
                                         xh_sb, xl_sb,
                                         slice(None), slice(tok0, tok0 + 512))
                                if qraw_pool:
                                    nc.gpsimd.tensor_copy(qraw[:, m, :], pq[:])
                                else:
                                    nc.scalar.copy(qraw[:, m, :], pq[:])
                            rope_q(qt_sb[:], qraw[:],
                                   slice(tok0, tok0 + 512))
                            qts.append(qt_sb)

                        # attention emission, software-pipelined `skew` deep.
                        # et planes are (kb0, causal, kb1) in one tile so both
                        # binmasks apply in a single DVE multiply.
                        PLANES = (0, 2, 1)  # plane j holds key-block PLANES[j]

                        def emit_scores(i):
                            g, lc = iters[i]
                            c = half * 4 + lc
                            et = wrk.tile([128, 3, 512], bf16,
                                          name=f"et{c}_{g}", tag="et",
                                          bufs=et_bufs)
                            for j, kb in enumerate(PLANES):
                                st = ps.tile([128, 512], f32,
                                             name=f"st{c}_{g}_{kb}", tag="ps")
                                nc.tensor.matmul(
                                    st[:],
                                    kt_tiles[g][:, c * 128 + kb * 128 :
                                                c * 128 + kb * 128 + 128],
                                    qts[g][:, :, lc * 128 : lc * 128 + 128],
                                    start=True,
                                    stop=True,
                                )
                                nc.scalar.activation(et[:, j, :], st[:],
                                                     exp_t, bias=0.0,
                                                     scale=SCALE8)
                            state[i] = (et, c, g, None)

                        def emit_masks(i):
                            # kb0 plane on (idle) Pool; causal on Pool early
                            # (while DVE chews the g3 rope), DVE once it's free
                            et, c, g, _ = state[i]
                            v = min(c, 2)
                            nc.gpsimd.tensor_mul(et[:, 0, :], et[:, 0, :],
                                                 bmp_sb[:, v, 0])
                            eng = nc.gpsimd if i < mpt else nc.vector
                            eng.tensor_mul(et[:, 1, :], et[:, 1, :],
                                           bmp_sb[:, v, 1])
                            state[i] = (et, c, g, None)

                        def emit_reduce(i):
                            et, c, g, _ = state.pop(i)
                            sums = ps.tile([128, 512], f32, name=f"sm{c}_{g}",
                                           tag="so", bufs=so_bufs)
                            outt = ps.tile([128, 512], f32, name=f"ot{c}_{g}",
                                           tag="so", bufs=so_bufs)
                            # consume the (DVE-masked) causal plane 1 last
                            for n, j in enumerate((0, 2, 1)):
                                nc.tensor.matmul(
                                    sums[:], ones_sb[:], et[:, j, :],
                                    start=(n == 0), stop=(n == 2),
                                )
                            for n, j in enumerate((0, 2, 1)):
                                kb = PLANES[j]
                                nc.tensor.matmul(
                                    outt[:],
                                    v_tiles[c + kb][:, g * 128 : (g + 1) * 128],
                                    et[:, j, :],
                                    start=(n == 0), stop=(n == 2),
                                )
                            rec = wrk.tile([128, 512], f32, name=f"rc{c}_{g}",
                                           tag="rec", bufs=2)
                            if c == 0:
                                sadj = wrk.tile([128, 512], f32,
                                                name=f"sa{g}", tag="sadj", bufs=2)
                                nc.vector.tensor_add(sadj[:], sums[:], corr_sb[:])
                                nc.vector.reciprocal_approx_fast(rec[:], sadj[:])
                            else:
                                nc.vector.reciprocal_approx_fast(rec[:], sums[:])
                            # hi/lo fp8 split of yt for the DoubleRow o-proj
                            ytf = wrk.tile([128, 512], bf16, name=f"ytf{g}_{c}",
                                           tag="ytf", bufs=3)
                            nc.vector.tensor_mul(ytf[:], outt[:], rec[:])
                            yth = ytp.tile([128, 512], fp8, name=f"yth{g}_{c}",
                                           tag="yt")
                            ytl = ytp.tile([128, 512], fp8, name=f"ytl{g}_{c}",
                                           tag="ytl")
                            nc.scalar.activation(yth[:], ytf[:], copy_t)
                            stt_eng = nc.vector if ytl_dve else nc.gpsimd
                            stt_eng.scalar_tensor_tensor(
                                ytl[:], ytf[:], 1.0, yth[:],
                                mybir.AluOpType.mult, mybir.AluOpType.subtract)
                            yts[(g, c % 4)] = (yth, ytl)

                        # Q proj g0..g2, then warmup scores so exp latency
                        # hides under the g3 projection matmuls; the previous
                        # half's O projection then fills the pipeline further.
                        qproj(0)
                        qproj(1)
                        qproj(2)
                        for i in range(skew):
                            emit_scores(i)
                            emit_masks(i)
                        qproj(3)
                        # 4 chunks at warmup, then 2 per iteration: fills the
                        # support-chain bubbles of early attention iterations
                        # without extending half-0 yt liveness past the yt
                        # buffer rotation
                        if prev_oproj is not None:
                            for ck in prev_oproj[:4]:
                                ck()
                        for i in range(skew, 16):
                            emit_scores(i)
                            if prev_oproj is not None:
                                j0 = 4 + 2 * (i - skew)
                                for ck in prev_oproj[j0 : j0 + 2]:
                                    ck()
                            emit_reduce(i - skew)
                            emit_masks(i)
                        prev_oproj = None
                        for i in range(16 - skew, 16):
                            emit_reduce(i)

                        # -- O projection: half 0's chunks interleave into
                        # half 1's attention; half 1's runs at the end --
                        if half == 0:
                            prev_oproj = oproj_chunks(half, yts)
                        else:
                            oproj_last(half, yts)

    nc.compile()
    return nc


def _prep_shared(wq, wk, wv, wo, rope_cache):
    """Host-side weight swizzles shared by all cores."""
    perm = np.concatenate([np.arange(0, 128, 2), np.arange(1, 128, 2)])

    wq_p = wq.reshape(NH, 128, C)[:, perm, :] * SW  # [h, d, C]
    wq_sw = np.ascontiguousarray(np.stack(_split8(np.ascontiguousarray(
        wq_p.reshape(NH, 128, 16, 128).transpose(0, 3, 2, 1)
    )), axis=2))  # [h, p, hi/lo, kc, n]

    wk_p = wk.reshape(NKV, 128, C)[:, perm, :] * SW
    wk_sw = np.ascontiguousarray(np.stack(_split8(np.ascontiguousarray(
        wk_p.reshape(NKV, 128, 16, 128).transpose(0, 3, 2, 1)
    )), axis=2))

    wv_sw = _split8(np.ascontiguousarray(
        (wv * SW).reshape(NKV * D, 16, 128).transpose(2, 1, 0)
    ))  # (hi, lo) [p, kc, n=512]

    # wo given [C, HD]; woT tiles [m, cs, p(d), hi/lo, n(c)] split to fp8 at
    # sigma~1 (x SW); the final o-evac divides by SV*SW.
    wo_sw = np.ascontiguousarray(np.stack(_split8(np.ascontiguousarray(
        wo.T.reshape(16, 128, 4, 512).transpose(0, 2, 1, 3) * SW
    )), axis=3))

    ones = np.ones((128, 128), dtype=bfloat16)

    # binary masks: key j (partition), query i (free, replicated over 4 heads)
    j = np.arange(128)[:, None]
    i = np.arange(128)[None, :]
    tri_kb0 = np.tile((j > i).astype(np.float32), (1, 4)).astype(bfloat16)
    tri_cau = np.tile((j <= i).astype(np.float32), (1, 4)).astype(bfloat16)
    zeros4 = np.zeros((128, 512), dtype=bfloat16)

    return wq_sw, wk_sw, wv_sw, wo_sw, ones, tri_kb0, tri_cau, zeros4


def _make_in_maps(x, wq, wk, wv, wo, rope_cache):
    (wq_sw, wk_sw, wv_sw, wo_sw, ones, tri_kb0, tri_cau, zeros4) = _prep_shared(
        wq, wk, wv, wo, rope_cache
    )

    in_maps = []
    for core in range(N_CORES):
        b, tq = divmod(core, 4)
        t0 = tq * TCORE
        boundary = t0 == 0

        # x^T with left halo, zero-padded below t=0
        xpad = np.zeros((C, TX), dtype=np.float32)
        lo = t0 - HALO
        src_lo = max(lo, 0)
        xpad[:, src_lo - lo :] = x[b, src_lo : t0 + TCORE, :].T
        xt_sw = _split8(np.ascontiguousarray(
            xpad.reshape(16, 128, TX).transpose(1, 0, 2)
        ))

        # combined rope tables: c1 = [cos | sin], c2 = [sin | cos] stacked on
        # partition halves (matching raw's [even | odd] layout)
        tglob = np.clip(np.arange(lo, t0 + TCORE), 0, T - 1)
        cosv = rope_cache[tglob, :, 0].T  # [64, TX]
        sinv = rope_cache[tglob, :, 1].T
        cs = np.empty((2, 128, TX), dtype=np.float32)
        cs[0, 0:64] = cosv
        cs[0, 64:128] = sinv
        cs[1, 0:64] = sinv
        cs[1, 64:128] = cosv

        # binmask plane pairs (kb0, causal) with kb0 variant by min(chunk, 2)
        bmp = np.empty((128, 3, 2, 512), dtype=bfloat16)
        for v in range(3):
            bmp[:, v, 0] = zeros4 if (boundary and v < 2) else tri_kb0
            bmp[:, v, 1] = tri_cau

        corr = np.full((128, 512), -128.0 if boundary else 0.0, dtype=np.float32)

        in_maps.append(
            {
                "xt_hi": xt_sw[0],
                "xt_lo": xt_sw[1],
                "wq_t": wq_sw,
                "wk_t": wk_sw,
                "wv_hi": wv_sw[0],
                "wv_lo": wv_sw[1],
                "wo_t": wo_sw,
                "cs_t": cs.astype(bfloat16),
                "bmp_t": bmp,
                "corr_t": corr,
                "ones_in": ones,
            }
        )
    return in_maps


def kernel(x, wq, wk, wv, wo, rope_cache):
    x = np.asarray(x, dtype=np.float32)
    wq = np.asarray(wq, dtype=np.float32)
    wk = np.asarray(wk, dtype=np.float32)
    wv = np.asarray(wv, dtype=np.float32)
    wo = np.asarray(wo, dtype=np.float32)
    rope_cache = np.asarray(rope_cache, dtype=np.float32)

    if "nc" not in _CACHE:
        _CACHE["nc"] = _build_nc()
    nc = _CACHE["nc"]

    in_maps = _make_in_maps(x, wq, wk, wv, wo, rope_cache)
    _CACHE["in_maps"] = in_maps

    res = bass_utils.run_bass_kernel_spmd(nc, in_maps, core_ids=list(range(N_CORES)))

    out = np.empty((B, T, C), dtype=np.float32)
    for core in range(N_CORES):
        b, tq = divmod(core, 4)
        out[b, tq * TCORE : (tq + 1) * TCORE, :] = res.results[core][
            "o_out"
        ].astype(np.float32)
    return out



# revision 88
# speedup vs baseline: 1.0522x; 1.0522x over previous
"""Trainium2 Bass kernel for nn_CleanAttention (sliding-window GQA attention).

Problem: x[2,4096,2048] -> qkv proj -> rope -> sliding-window (256) attention
(16 q heads, 4 kv heads, d=128) -> o proj.

Sharding: 8 cores = batch(2) x token-quarters(4). Each core computes all 16
heads for its 1024 tokens, using a 256-token key/value halo on the left.
Outputs concatenate: no inter-core reduction.

v3 dataflow (per core): all four projections run as split-fp8 DoubleRow
matmuls (operands held as fp8e4m3 hi+lo pairs; out = hi*hi + hi*lo + lo*hi,
each term a K=256-per-instruction DoubleRow chain at 4x bf16 throughput =>
net 0.75x the bf16 cost with ~bf16 accuracy). Weights are globally scaled to
sigma~1 host-side (SW=sqrt(2048)) so the fp8 lo-residuals stay out of the
denormal floor; the scales fold into the exp scale, the V evacuation
(SV/SW), and the final output copy (1/(SV*SW)). x and all weights are split
host-side for free; yt is split on-device (DVE mul -> ACT fp8 cast -> DVE
residual via scalar_tensor_tensor).

  x_hi/lo   fp8 [128p, 16kc, 1280t] resident
  V   = x @ wv.T   10 x [128t, 512hd] bf16 (ACT evac scales by SV/SW)
  K^T = wk_p @ x.T (+rope)   4 x [128d, 1280t] bf16
  Q^T = wq_p @ x.T (+rope)   per (half,g): [128d, 4h, 512t] bf16
  attention (i = (g,chunk)), software-pipelined `skew` deep, bf16 matmuls:
    S^T(i) = K^T_blk @ Q^T; E^T = exp(S^T * SCALE/SW^2) (ACT);
    binmasks on Pool; sums/outT matmuls; rec = 1/sums (DVE);
    ytf = outT*rec (DVE bf16) -> yth fp8 (ACT) + ytl fp8 (DVE)
  o = yt @ wo.T as DoubleRow pairs of heads; half 0's o-proj is chopped into
  16 (csx,lc) chunks interleaved one-two per attention iteration of half 1
  (fills PE bubbles left by the ACT/DVE/Pool support chain); half 1's runs
  at the end with wo prefetched one csx ahead.

DMA: everything except nothing rides the SP HWDGE queue in PE-consumption
order (the ACT queue head-of-line-blocks its own activations); x moves in
>=512B-run chunks; wq/wk hi+lo packed per head/group for one DMA each; wo in
4 tiles per csx.

RoPE: wq/wk rows host-permuted per head to [even dims | odd dims]; rotation
is 6 DVE ops on bf16; Q-side tables are stride-0 broadcasts of slices of the
K tables (no separate Q tables).
Boundary masking: halo x is zero => K=0 => scores=0 => exp=1; binmasks zero
the kb0/causal planes, and the unmasked kb1 plane of chunk 0 is corrected by
subtracting 128 from the softmax denominator (corr plane, nonzero only on
cores 0 and 4).
"""

import math

import numpy as np
from ml_dtypes import bfloat16

import concourse.bass as bass
import concourse.mybir as mybir
import concourse.tile as tile
from concourse import bacc
from concourse import bass_utils

B, T, C = 2, 4096, 2048
NH, NKV, D = 16, 4, 128
WINDOW = 256
N_CORES = 8
TCORE = 1024  # own tokens per core
HALO = 256
TX = TCORE + HALO  # 1280
NG = 4  # kv groups
GH = 4  # q heads per group
NCHUNK = 8  # query chunks of 128 per core
SCALE = 1.0 / math.sqrt(D)

f32 = mybir.dt.float32
bf16 = mybir.dt.bfloat16
fp8 = mybir.dt.float8e4
np8 = mybir.dt.np(fp8)
DR = mybir.MatmulPerfMode.DoubleRow

# Global operand scaling for fp8: weights scaled to sigma~1 (x already is).
# wq,wk scaled by SW => scores scaled SW^2, folded into the exp scale.
# wv scaled by SW => V-evac rescales by SV/SW so yt = SV*y (sigma~1 for its
# fp8 split); wo scaled by SW for its split; final evac divides by SV*SW.
SW = float(np.sqrt(2048.0))
SV = 16.0
SCALE8 = SCALE / (SW * SW)

_CACHE = {}


def _split8(a):
    """Return (hi, lo) float8_e4m3 split of array a (hi+lo ~= a)."""
    hi = a.astype(np8)
    lo = (a.astype(np.float32) - hi.astype(np.float32)).astype(np8)
    return hi, lo


def _build_nc(repeat=1, st_bufs=6, so_bufs=2, et_bufs=5, wq_bufs=4, wo_bufs=9,
              rt_bufs=3, rawq_bufs=2, yt_bufs=19, osb_bufs=4, skew=4, mpt=16,
              ytl_dve=True, wo_merge=True, ph0_pool=False, qraw_pool=False,
              e01t=0, wq_act=False, wo_bufs2=3):
    nc = bacc.Bacc("TRN2", target_bir_lowering=False, debug=False)

    xt_hi = nc.dram_tensor("xt_hi", [128, 16, TX], fp8, kind="ExternalInput")
    xt_lo = nc.dram_tensor("xt_lo", [128, 16, TX], fp8, kind="ExternalInput")
    # hi/lo splits packed on axis 1 => one DMA per head/group
    wq_t = nc.dram_tensor("wq_t", [NH, 128, 2, 16, 128], fp8, kind="ExternalInput")
    wk_t = nc.dram_tensor("wk_t", [NKV, 128, 2, 16, 128], fp8, kind="ExternalInput")
    wv_hi = nc.dram_tensor("wv_hi", [128, 16, 512], fp8, kind="ExternalInput")
    wv_lo = nc.dram_tensor("wv_lo", [128, 16, 512], fp8, kind="ExternalInput")
    wo_t = nc.dram_tensor("wo_t", [16, 4, 128, 2, 512], fp8, kind="ExternalInput")
    cs_t = nc.dram_tensor("cs_t", [2, 128, TX], bf16, kind="ExternalInput")
    bmp_t = nc.dram_tensor("bmp_t", [128, 3, 2, 512], bf16, kind="ExternalInput")
    corr_t = nc.dram_tensor("corr_t", [128, 512], f32, kind="ExternalInput")
    ones_in = nc.dram_tensor("ones_in", [128, 128], bf16, kind="ExternalInput")
    o_out = nc.dram_tensor("o_out", [TCORE, C], bf16, kind="ExternalOutput")

    exp_t = mybir.ActivationFunctionType.Exp
    copy_t = mybir.ActivationFunctionType.Copy

    with tile.TileContext(nc) as tc:
        with (
            tc.sbuf_pool(name="fixed", bufs=1) as fixed,
            tc.sbuf_pool(name="xtp", bufs=1) as xtp,
            tc.sbuf_pool(name="ktp", bufs=1) as ktp,
            tc.sbuf_pool(name="vp", bufs=1) as vp,
            tc.sbuf_pool(name="ropetmp", bufs=1) as ropetmp,
            tc.sbuf_pool(name="wqp", bufs=wq_bufs) as wqp,
            tc.psum_pool(name="ps", bufs=st_bufs) as ps,
        ):
            # --- fixed small tables (order = DMA priority; compute-critical
            # loads for phase 0 are issued inside the rep loop before these
            # on the first pass via emission order) ---
            cos_sb = fixed.tile([128, TX], bf16)
            sin_sb = fixed.tile([128, TX], bf16)
            bmp_sb = fixed.tile([128, 3, 2, 512], bf16)
            corr_sb = fixed.tile([128, 512], f32)
            ones_sb = fixed.tile([128, 128], bf16)

            def load_fixed():
                # on the SP queue AFTER the phase-0 critical feed: queue
                # order (not emission position) is what the scheduler keeps
                nc.sync.dma_start(cos_sb[:], cs_t[0])
                nc.sync.dma_start(sin_sb[:], cs_t[1])
                nc.sync.dma_start(bmp_sb[:], bmp_t[:])
                nc.sync.dma_start(corr_sb[:], corr_t[:])
                nc.sync.dma_start(ones_sb[:], ones_in[:])

            def rope(dst, raw, c1_ap, c2_ap, width):
                # raw rows [0:64]=even dims e, [64:128]=odd dims o (bf16 sbuf)
                # c1 = [cos (top) | sin (bottom)], c2 = [sin | cos]
                # dst[0:64] = e*cos - o*sin ; dst[64:128] = e*sin + o*cos
                # (tensor_tensor inputs must share the start partition, so
                # each product lands in a base-0 temp tile first)
                t1 = ropetmp.tile([64, 2048], bf16, name="t1", tag="rt",
                                  bufs=rt_bufs)
                t2 = ropetmp.tile([64, 2048], bf16, name="t2", tag="rt",
                                  bufs=rt_bufs)
                nc.vector.tensor_mul(t1[:, :width], raw[0:64], c1_ap[0:64])
                nc.vector.tensor_mul(t2[:, :width], raw[64:128], c1_ap[64:128])
                nc.vector.tensor_sub(dst[0:64], t1[:, :width], t2[:, :width])
                t3 = ropetmp.tile([64, 2048], bf16, name="t3", tag="rt",
                                  bufs=rt_bufs)
                t4 = ropetmp.tile([64, 2048], bf16, name="t4", tag="rt",
                                  bufs=rt_bufs)
                nc.vector.tensor_mul(t3[:, :width], raw[0:64], c2_ap[0:64])
                nc.vector.tensor_mul(t4[:, :width], raw[64:128], c2_ap[64:128])
                nc.vector.tensor_add(dst[64:128], t3[:, :width], t4[:, :width])

            def rope_q(dst, raw, sl):
                # dst/raw: [128, 4, 512] APs; rope tables are slices of the
                # K tables broadcast (stride-0) over the 4-head dim
                def bc(tab, p0):
                    return tab[p0 : p0 + 64, sl][:, None, :].to_broadcast(
                        [64, 4, 512])

                ts_ = [ropetmp.tile([64, 2048], bf16, name=f"t{n}q", tag="rt",
                                    bufs=rt_bufs) for n in range(4)]
                tv = [t[:].rearrange("p (a b) -> p a b", a=4) for t in ts_]
                nc.vector.tensor_mul(tv[0], raw[0:64], bc(cos_sb, 0))
                nc.vector.tensor_mul(tv[1], raw[64:128], bc(cos_sb, 64))
                nc.vector.tensor_sub(dst[0:64], tv[0], tv[1])
                nc.vector.tensor_mul(tv[2], raw[0:64], bc(sin_sb, 0))
                nc.vector.tensor_mul(tv[3], raw[64:128], bc(sin_sb, 64))
                nc.vector.tensor_add(dst[64:128], tv[2], tv[3])

            for rep in range(repeat):
                xh_sb = xtp.tile([128, 16, TX], fp8, name="xh_sb", tag="xt")
                xl_sb = xtp.tile([128, 16, TX], fp8, name="xl_sb", tag="xtl")
                kt_tiles = [
                    ktp.tile([128, TX], bf16, name=f"ktg{g}", tag=f"ktg{g}")
                    for g in range(NG)
                ]
                v_tiles = [
                    vp.tile([128, 512], bf16, name=f"vtb{tb}", tag=f"vtb{tb}")
                    for tb in range(10)
                ]

                # ---- phase 0: V and K projections ----
                with (
                    tc.sbuf_pool(name="wvp", bufs=1) as wvp,
                    tc.sbuf_pool(name="wkp", bufs=2) as wkp,
                    tc.sbuf_pool(name="krawp", bufs=4) as krawp,
                ):
                    # DMA priority order: hi operands + first x chunks feed
                    # the first matmul terms; lo operands follow; tables and
                    # Q-side constants come later.
                    wvh_sb = wvp.tile([128, 16, 512], fp8, name="wvh")
                    wvl_sb = wvp.tile([128, 16, 512], fp8, name="wvl")
                    # SP queue in PE consumption order (kproj first: each wk
                    # DMA is 728ns per 2.6us of PE work unlocked); few large
                    # DMAs since HWDGE costs 625ns fixed per transfer
                    wk_sbs = []
                    for g in range(NG):
                        wk_sbs.append(wkp.tile([128, 2, 16, 128], fp8,
                                               name=f"wkg{g}", tag="wk",
                                               bufs=NG))
                    nc.sync.dma_start(xh_sb[:, :, 0:128], xt_hi[:, :, 0:128])
                    nc.sync.dma_start(wk_sbs[0][:, 0], wk_t[0, :, 0])
                    nc.sync.dma_start(wk_sbs[0][:, 1], wk_t[0, :, 1])
                    for g in range(1, NG):
                        nc.sync.dma_start(wk_sbs[g][:], wk_t[g])
                    nc.sync.dma_start(xl_sb[:, :, 0:128], xt_lo[:, :, 0:128])
                    nc.sync.dma_start(xh_sb[:, :, 128:640], xt_hi[:, :, 128:640])
                    nc.sync.dma_start(xl_sb[:, :, 128:640], xt_lo[:, :, 128:640])
                    nc.sync.dma_start(wvh_sb[:], wv_hi[:])
                    nc.sync.dma_start(wvl_sb[:], wv_lo[:])
                    nc.sync.dma_start(xh_sb[:, :, 640:1152], xt_hi[:, :, 640:1152])
                    nc.sync.dma_start(xl_sb[:, :, 640:1152], xt_lo[:, :, 640:1152])
                    nc.sync.dma_start(xh_sb[:, :, 1152:TX], xt_hi[:, :, 1152:TX])
                    nc.sync.dma_start(xl_sb[:, :, 1152:TX], xt_lo[:, :, 1152:TX])

                    def dr_terms(p_ap, lhs_hi, lhs_lo, rhs_hi, rhs_lo, lslc, rslc):
                        """24 DoubleRow matmuls: hi*hi + hi*lo + lo*hi."""
                        terms = ((lhs_hi, rhs_hi), (lhs_hi, rhs_lo),
                                 (lhs_lo, rhs_hi))
                        for ti, (la, rb) in enumerate(terms):
                            for kk in range(8):
                                k2 = slice(2 * kk, 2 * kk + 2)
                                nc.tensor.matmul(
                                    p_ap,
                                    la[:, k2, lslc],
                                    rb[:, k2, rslc],
                                    start=(ti == 0 and kk == 0),
                                    stop=(ti == 2 and kk == 7),
                                    perf_mode=DR,
                                )

                    def vproj(tb):
                        pv = ps.tile([128, 512], f32, name=f"pv{tb}", tag="ps")
                        dr_terms(pv[:], xh_sb, xl_sb, wvh_sb, wvl_sb,
                                 slice(tb * 128, (tb + 1) * 128), slice(None))
                        if ph0_pool:
                            nc.gpsimd.tensor_scalar_mul(v_tiles[tb][:], pv[:],
                                                        SV / SW)
                        else:
                            nc.scalar.activation(v_tiles[tb][:], pv[:], copy_t,
                                                 bias=0.0, scale=SV / SW)

                    kraws = {}

                    def kproj_all(ts, tw):
                        # per-term emission across all groups, matching DMA
                        # arrival: wk_hi*xh (4g), wk_lo*xh (4g), wk_hi*xl (4g)
                        rs = slice(ts, ts + tw)
                        pks = {}
                        for g in range(NG):
                            pks[g] = ps.tile([128, 512], f32,
                                             name=f"pk{g}_{ts}", tag="ps")
                        for ti, (wi, xa) in enumerate(
                                ((0, xh_sb), (1, xh_sb), (0, xl_sb))):
                            for g in range(NG):
                                for kk in range(8):
                                    k2 = slice(2 * kk, 2 * kk + 2)
                                    nc.tensor.matmul(
                                        pks[g][:, :tw],
                                        wk_sbs[g][:, wi, k2, :],
                                        xa[:, k2, rs],
                                        start=(ti == 0 and kk == 0),
                                        stop=(ti == 2 and kk == 7),
                                        perf_mode=DR,
                                    )
                                if ti == 2:
                                    if g not in kraws:
                                        kraws[g] = krawp.tile(
                                            [128, TX], bf16,
                                            name=f"kraw{g}", tag="kraw")
                                    if ph0_pool:
                                        nc.gpsimd.tensor_copy(
                                            kraws[g][:, rs], pks[g][:, :tw])
                                    else:
                                        nc.scalar.copy(kraws[g][:, rs],
                                                       pks[g][:, :tw])

                    # interleave with xt chunk arrival order
                    kproj_all(0, 128)
                    kproj_all(128, 512)
                    for tb in (0, 1, 2, 3):
                        vproj(tb)
                    if rep == 0:
                        load_fixed()
                    kproj_all(640, 512)
                    for tb in (4, 5, 6, 7):
                        vproj(tb)
                    kproj_all(1152, 128)
                    for tb in (8, 9):
                        vproj(tb)
                    for g in range(NG):
                        rope(kt_tiles[g], kraws[g], cos_sb, sin_sb, TX)

                # ---- main: per half: Q proj + attention + O proj ----
                with (
                    tc.sbuf_pool(name="qtp", bufs=5) as qtp,
                    tc.sbuf_pool(name="ytp", bufs=yt_bufs) as ytp,
                    tc.sbuf_pool(name="wop", bufs=wo_bufs) as wop,
                    tc.sbuf_pool(name="wrk", bufs=1) as wrk,
                ):
                    def o_evac(hh, csx, lc, pos_lc, split=False):
                        osb = wrk.tile([128, 512], bf16,
                                       name=f"osb{hh}_{csx}_{lc}",
                                       tag="osb", bufs=osb_bufs)
                        r0 = (hh * 4 + lc) * 128
                        c0 = csx * 512
                        # split=True pipelines copy+store in halves to cut
                        # the end-of-kernel drain latency
                        parts = ((0, 256), (256, 512)) if split else ((0, 512),)
                        for a, b in parts:
                            nc.scalar.activation(osb[:, a:b], pos_lc[:, a:b],
                                                 copy_t, bias=0.0,
                                                 scale=1.0 / (SV * SW))
                            nc.sync.dma_start(
                                o_out[r0 : r0 + 128, c0 + a : c0 + b],
                                osb[:, a:b],
                            )

                    def wo_load(hh, csx):
                        wo_sbs = []
                        for q in range(4):
                            wo_sb = wop.tile([128, 2, 2, 2, 512], fp8,
                                             name=f"wo{hh}_q{q}_{csx}",
                                             tag="wo", bufs=2 * wo_bufs2)
                            nc.sync.dma_start(
                                wo_sb[:],
                                wo_t[4 * q : 4 * q + 4, csx].rearrange(
                                    "(a m) p l n -> p a m l n", m=2
                                ),
                            )
                            wo_sbs.extend(wo_sb[:, a] for a in range(2))
                        return wo_sbs

                    def o_mms(hh, hyts, wo_sbs, po, csx, lc, evac_inline,
                              evac_split=False):
                        # 24 DR matmuls: mp pairs heads (2mp, 2mp+1); terms
                        # y_hi*wo_hi, y_hi*wo_lo, y_lo*wo_hi. mp-major so
                        # each wo tile is fully consumed early, freeing its
                        # buffer for the next csx's prefetch DMA
                        for mp in range(8):
                            for ti in range(3):
                                m0 = 2 * mp
                                yh, yl = hyts[(m0 // 4, lc)]
                                ya = yl if ti == 2 else yh
                                hl = 1 if ti == 1 else 0
                                nc.tensor.matmul(
                                    po[:],
                                    ya[:, (m0 % 4) * 128 : (m0 % 4) * 128 + 256]
                                    .rearrange("p (k m) -> p k m", k=2),
                                    wo_sbs[mp][:, :, hl, :],
                                    start=(mp == 0 and ti == 0),
                                    stop=(mp == 7 and ti == 2),
                                    perf_mode=DR,
                                )
                        if evac_inline:
                            o_evac(hh, csx, lc, po, split=evac_split)

                    def oproj_chunks(hh, hyts):
                        # 16 closures, one (csx, lc) chunk each, emitted one
                        # per attention iteration of the NEXT half so the PE
                        # fills support-chain bubbles; wo prefetch 1 csx ahead
                        wo_map = {}

                        def chunk(j):
                            csx, lc = divmod(j, 4)
                            if j == 0:
                                wo_map[0] = wo_load(hh, 0)
                            if lc == 2 and csx + 1 < 4:
                                wo_map[csx + 1] = wo_load(hh, csx + 1)
                            po = ps.tile([128, 512], f32,
                                         name=f"po{hh}_{csx}_{lc}", tag="ps")
                            o_mms(hh, hyts, wo_map[csx], po, csx, lc, True)

                        return [lambda j=j: chunk(j) for j in range(16)]

                    def oproj_last(hh, hyts, pre):
                        sbs = dict(pre)
                        if 0 not in sbs:
                            sbs[0] = wo_load(hh, 0)
                        for csx in range(4):
                            if csx + 1 < 4 and csx + 1 not in sbs:
                                sbs[csx + 1] = wo_load(hh, csx + 1)
                            for lc in range(4):
                                po = ps.tile([128, 512], f32,
                                             name=f"po{hh}_{csx}_{lc}",
                                             tag="ps")
                                o_mms(hh, hyts, sbs[csx], po, csx, lc, True,
                                      evac_split=False)

                    prev_oproj = None
                    final_wo = {}
                    for half in range(2):
                        tok0 = 256 + half * 512  # local token offset of queries
                        iters = [(g, lc) for g in range(NG) for lc in range(4)]
                        state = {}  # i -> (et, c, g)
                        qts = []
                        yts = {}

                        def qproj(g):
                            qt_sb = qtp.tile([128, GH, 512], bf16,
                                             name=f"qt{half}_{g}", tag="qt")
                            qraw = ropetmp.tile([128, 4, 512], bf16,
                                                name=f"qraw{g}", tag="qraw",
                                                bufs=rawq_bufs)
                            for m in range(GH):
                                h = g * GH + m
                                wq_sb = wqp.tile([128, 2, 16, 128], fp8,
                                                 name=f"wqh{h}", tag="wq")
                                (nc.scalar if wq_act else nc.sync).dma_start(
                                    wq_sb[:], wq_t[h])
                                pq = ps.tile([128, 512], f32, name=f"pq{h}",
                                             tag="ps")
                                dr_terms(pq[:], wq_sb[:, 0], wq_sb[:, 1],
                                         xh_sb, xl_sb,
                                         slice(None), slice(tok0, tok0 + 512))
                                if qraw_pool:
                                    nc.gpsimd.tensor_copy(qraw[:, m, :], pq[:])
                                else:
                                    nc.scalar.copy(qraw[:, m, :], pq[:])
                            rope_q(qt_sb[:], qraw[:],
                                   slice(tok0, tok0 + 512))
                            qts.append(qt_sb)

                        # attention emission, software-pipelined `skew` deep.
                        # et planes are (kb0, causal, kb1) in one tile so both
                        # binmasks apply in a single DVE multiply.
                        PLANES = (0, 2, 1)  # plane j holds key-block PLANES[j]

                        def emit_scores(i):
                            g, lc = iters[i]
                            c = half * 4 + lc
                            et = wrk.tile([128, 3, 512], bf16,
                                          name=f"et{c}_{g}", tag="et",
                                          bufs=et_bufs)
                            for j, kb in enumerate(PLANES):
                                st = ps.tile([128, 512], f32,
                                             name=f"st{c}_{g}_{kb}", tag="ps")
                                nc.tensor.matmul(
                                    st[:],
                                    kt_tiles[g][:, c * 128 + kb * 128 :
                                                c * 128 + kb * 128 + 128],
                                    qts[g][:, :, lc * 128 : lc * 128 + 128],
                                    start=True,
                                    stop=True,
                                )
                                nc.scalar.activation(et[:, j, :], st[:],
                                                     exp_t, bias=0.0,
                                                     scale=SCALE8)
                            state[i] = (et, c, g, None)

                        def emit_masks(i):
                            # kb0 plane on (idle) Pool; causal on Pool early
                            # (while DVE chews the g3 rope), DVE once it's free
                            et, c, g, _ = state[i]
                            v = min(c, 2)
                            nc.gpsimd.tensor_mul(et[:, 0, :], et[:, 0, :],
                                                 bmp_sb[:, v, 0])
                            eng = nc.gpsimd if i < mpt else nc.vector
                            eng.tensor_mul(et[:, 1, :], et[:, 1, :],
                                           bmp_sb[:, v, 1])
                            state[i] = (et, c, g, None)

                        def emit_reduce(i):
                            et, c, g, _ = state.pop(i)
                            sums = ps.tile([128, 512], f32, name=f"sm{c}_{g}",
                                           tag="so", bufs=so_bufs)
                            outt = ps.tile([128, 512], f32, name=f"ot{c}_{g}",
                                           tag="so", bufs=so_bufs)
                            # consume the (DVE-masked) causal plane 1 last
                            for n, j in enumerate((0, 2, 1)):
                                nc.tensor.matmul(
                                    sums[:], ones_sb[:], et[:, j, :],
                                    start=(n == 0), stop=(n == 2),
                                )
                            for n, j in enumerate((0, 2, 1)):
                                kb = PLANES[j]
                                nc.tensor.matmul(
                                    outt[:],
                                    v_tiles[c + kb][:, g * 128 : (g + 1) * 128],
                                    et[:, j, :],
                                    start=(n == 0), stop=(n == 2),
                                )
                            rec = wrk.tile([128, 512], f32, name=f"rc{c}_{g}",
                                           tag="rec", bufs=2)
                            if c == 0:
                                sadj = wrk.tile([128, 512], f32,
                                                name=f"sa{g}", tag="sadj", bufs=2)
                                nc.vector.tensor_add(sadj[:], sums[:], corr_sb[:])
                                nc.vector.reciprocal_approx_fast(rec[:], sadj[:])
                            else:
                                nc.vector.reciprocal_approx_fast(rec[:], sums[:])
                            # hi/lo fp8 split of yt for the DoubleRow o-proj
                            ytf = wrk.tile([128, 512], bf16, name=f"ytf{g}_{c}",
                                           tag="ytf", bufs=3)
                            nc.vector.tensor_mul(ytf[:], outt[:], rec[:])
                            yth = ytp.tile([128, 512], fp8, name=f"yth{g}_{c}",
                                           tag="yt")
                            ytl = ytp.tile([128, 512], fp8, name=f"ytl{g}_{c}",
                                           tag="ytl")
                            nc.scalar.activation(yth[:], ytf[:], copy_t)
                            stt_eng = nc.vector if ytl_dve else nc.gpsimd
                            stt_eng.scalar_tensor_tensor(
                                ytl[:], ytf[:], 1.0, yth[:],
                                mybir.AluOpType.mult, mybir.AluOpType.subtract)
                            yts[(g, c % 4)] = (yth, ytl)

                        # Q proj g0..g2, then warmup scores so exp latency
                        # hides under the g3 projection matmuls; the previous
                        # half's O projection then fills the pipeline further.
                        qproj(0)
                        qproj(1)
                        qproj(2)
                        for i in range(2):
                            emit_scores(i)
                            emit_masks(i)
                        qproj(3)
                        for i in range(2, skew):
                            emit_scores(i)
                            emit_masks(i)
                        # 4 chunks at warmup, then 2 per iteration: fills the
                        # support-chain bubbles of early attention iterations
                        # without extending half-0 yt liveness past the yt
                        # buffer rotation
                        if prev_oproj is not None:
                            for ck in prev_oproj[:4]:
                                ck()
                        for i in range(skew, 16):
                            emit_scores(i)
                            if prev_oproj is not None:
                                j0 = 4 + 2 * (i - skew)
                                for ck in prev_oproj[j0 : j0 + 2]:
                                    ck()
                            if half == 1 and i in (12, 14):
                                # prefetch the final o-proj's first wo tiles
                                # while the SP queue is otherwise idle
                                final_wo[(i - 12) // 2] = wo_load(1, (i - 12) // 2)
                            emit_reduce(i - skew)
                            emit_masks(i)
                        prev_oproj = None
                        for i in range(16 - skew, 16):
                            emit_reduce(i)

                        # -- O projection: half 0's chunks interleave into
                        # half 1's attention; half 1's runs at the end --
                        if half == 0:
                            prev_oproj = oproj_chunks(half, yts)
                        else:
                            oproj_last(half, yts, final_wo)

    nc.compile()
    return nc


def _prep_shared(wq, wk, wv, wo, rope_cache):
    """Host-side weight swizzles shared by all cores."""
    perm = np.concatenate([np.arange(0, 128, 2), np.arange(1, 128, 2)])

    wq_p = wq.reshape(NH, 128, C)[:, perm, :] * SW  # [h, d, C]
    wq_sw = np.ascontiguousarray(np.stack(_split8(np.ascontiguousarray(
        wq_p.reshape(NH, 128, 16, 128).transpose(0, 3, 2, 1)
    )), axis=2))  # [h, p, hi/lo, kc, n]

    wk_p = wk.reshape(NKV, 128, C)[:, perm, :] * SW
    wk_sw = np.ascontiguousarray(np.stack(_split8(np.ascontiguousarray(
        wk_p.reshape(NKV, 128, 16, 128).transpose(0, 3, 2, 1)
    )), axis=2))

    wv_sw = _split8(np.ascontiguousarray(
        (wv * SW).reshape(NKV * D, 16, 128).transpose(2, 1, 0)
    ))  # (hi, lo) [p, kc, n=512]

    # wo given [C, HD]; woT tiles [m, cs, p(d), hi/lo, n(c)] split to fp8 at
    # sigma~1 (x SW); the final o-evac divides by SV*SW.
    wo_sw = np.ascontiguousarray(np.stack(_split8(np.ascontiguousarray(
        wo.T.reshape(16, 128, 4, 512).transpose(0, 2, 1, 3) * SW
    )), axis=3))

    ones = np.ones((128, 128), dtype=bfloat16)

    # binary masks: key j (partition), query i (free, replicated over 4 heads)
    j = np.arange(128)[:, None]
    i = np.arange(128)[None, :]
    tri_kb0 = np.tile((j > i).astype(np.float32), (1, 4)).astype(bfloat16)
    tri_cau = np.tile((j <= i).astype(np.float32), (1, 4)).astype(bfloat16)
    zeros4 = np.zeros((128, 512), dtype=bfloat16)

    return wq_sw, wk_sw, wv_sw, wo_sw, ones, tri_kb0, tri_cau, zeros4


def _make_in_maps(x, wq, wk, wv, wo, rope_cache):
    (wq_sw, wk_sw, wv_sw, wo_sw, ones, tri_kb0, tri_cau, zeros4) = _prep_shared(
        wq, wk, wv, wo, rope_cache
    )

    in_maps = []
    for core in range(N_CORES):
        b, tq = divmod(core, 4)
        t0 = tq * TCORE
        boundary = t0 == 0

        # x^T with left halo, zero-padded below t=0
        xpad = np.zeros((C, TX), dtype=np.float32)
        lo = t0 - HALO
        src_lo = max(lo, 0)
        xpad[:, src_lo - lo :] = x[b, src_lo : t0 + TCORE, :].T
        xt_sw = _split8(np.ascontiguousarray(
            xpad.reshape(16, 128, TX).transpose(1, 0, 2)
        ))

        # combined rope tables: c1 = [cos | sin], c2 = [sin | cos] stacked on
        # partition halves (matching raw's [even | odd] layout)
        tglob = np.clip(np.arange(lo, t0 + TCORE), 0, T - 1)
        cosv = rope_cache[tglob, :, 0].T  # [64, TX]
        sinv = rope_cache[tglob, :, 1].T
        cs = np.empty((2, 128, TX), dtype=np.float32)
        cs[0, 0:64] = cosv
        cs[0, 64:128] = sinv
        cs[1, 0:64] = sinv
        cs[1, 64:128] = cosv

        # binmask plane pairs (kb0, causal) with kb0 variant by min(chunk, 2)
        bmp = np.empty((128, 3, 2, 512), dtype=bfloat16)
        for v in range(3):
            bmp[:, v, 0] = zeros4 if (boundary and v < 2) else tri_kb0
            bmp[:, v, 1] = tri_cau

        corr = np.full((128, 512), -128.0 if boundary else 0.0, dtype=np.float32)

        in_maps.append(
            {
                "xt_hi": xt_sw[0],
                "xt_lo": xt_sw[1],
                "wq_t": wq_sw,
                "wk_t": wk_sw,
                "wv_hi": wv_sw[0],
                "wv_lo": wv_sw[1],
                "wo_t": wo_sw,
                "cs_t": cs.astype(bfloat16),
                "bmp_t": bmp,
                "corr_t": corr,
                "ones_in": ones,
            }
        )
    return in_maps


def kernel(x, wq, wk, wv, wo, rope_cache):
    x = np.asarray(x, dtype=np.float32)
    wq = np.asarray(wq, dtype=np.float32)
    wk = np.asarray(wk, dtype=np.float32)
    wv = np.asarray(wv, dtype=np.float32)
    wo = np.asarray(wo, dtype=np.float32)
    rope_cache = np.asarray(rope_cache, dtype=np.float32)

    if "nc" not in _CACHE:
        _CACHE["nc"] = _build_nc()
    nc = _CACHE["nc"]

    in_maps = _make_in_maps(x, wq, wk, wv, wo, rope_cache)
    _CACHE["in_maps"] = in_maps

    res = bass_utils.run_bass_kernel_spmd(nc, in_maps, core_ids=list(range(N_CORES)))

    out = np.empty((B, T, C), dtype=np.float32)
    for core in range(N_CORES):
        b, tq = divmod(core, 4)
        out[b, tq * TCORE : (tq + 1) * TCORE, :] = res.results[core][
            "o_out"
        ].astype(np.float32)
    return out

